# revision 4
# baseline (speedup 1.0000x reference)
"""Adaptive average pool 2D (16, 256, 224, 224) -> (16, 256, 7, 7) on 8 TRN2 NeuronCores.

224 / 7 = 32 exactly, so every adaptive-pool window is a non-overlapping
32x32 block: out[b, c, o, p] = mean(x[b, c, 32o:32o+32, 32p:32p+32]).

Sharding: pure data parallel over batch — 2 batches (512 (b, c) images) per core.

Per-core layout: each of the 512 images is 224*224 contiguous floats in HBM.
A load tile holds one 32-row h-block for 128 images: [128 partitions, 32*224],
i.e. 28 KiB contiguous per partition per DMA (maximally DMA-efficient, 3.7 MB
per dma_start). One VectorE reduce_sum over the innermost (h, wi) axes of the
view [128, 7(wblock), 32(h), 32(wi)] produces the 7 pooled sums per image per
h-block, written into a persistent [128, 4*49] accumulator. ScalarE applies
the 1/1024 scale once, then a single DMA stores all 512*49 outputs.

Raw Bass (no Tile): the walrus DMA lowering accepts only one sync-wait per
DMACopy, so waits are emitted as standalone sequencer waits and each DMA
carries exactly one semaphore update.
"""

import numpy as np
from contextlib import ExitStack

import concourse.bass as bass
from concourse import mybir
from concourse.bass_utils import run_bass_kernel_spmd

N_CORES = 8
B, C, H, W = 16, 256, 224, 224
HO = WO = 7
BH, BW = H // HO, W // WO            # 32, 32
IMGS = (B // N_CORES) * C            # 512 images per core
PG = IMGS // 128                     # 4 partition groups of 128 images
ROW = BH * W                         # 7168 floats per (image, h-block)
NTILES = PG * HO                     # 28 load tiles per core
NBUF = 4                             # load tile slots (double++ buffering)

_CACHE = {}


def build_nc():
    nc = bass.Bass("TRN2", debug=False, num_devices=N_CORES)
    x = nc.dram_tensor("x", [IMGS, H * W], mybir.dt.float32, kind="ExternalInput")
    out = nc.dram_tensor("out", [IMGS, HO * WO], mybir.dt.float32, kind="ExternalOutput")
    xa, oa = x.ap(), out.ap()
    # dst view [128 part, (g, j)]: out[(g*128+p), j]
    oav = oa.rearrange("(g p) j -> p g j", g=PG)

    with ExitStack() as ctx:
        tiles = ctx.enter_context(nc.sbuf_tensor([128, NBUF * ROW], mybir.dt.float32))
        ob = ctx.enter_context(nc.sbuf_tensor([128, PG * HO * WO], mybir.dt.float32))
        slot_sem = [ctx.enter_context(nc.semaphore(f"slot{j}")) for j in range(NBUF)]
        red_done = ctx.enter_context(nc.semaphore("red_done"))
        act_done = ctx.enter_context(nc.semaphore("act_done"))
        out_sem = ctx.enter_context(nc.semaphore("out_sem"))
        block = ctx.enter_context(nc.Block())

        def src(i):
            g, hb = divmod(i, HO)
            return xa[g * 128:(g + 1) * 128, hb * ROW:(hb + 1) * ROW]

        def slot(j):
            return tiles[:, j * ROW:(j + 1) * ROW]

        @block.sync
        def _(sync):
            for i in range(NTILES):
                j = i % NBUF
                if i >= NBUF:
                    # slot reuse: wait for the reduce that read this slot
                    sync.wait_ge(red_done, i - NBUF + 1)
                sync.dma_start(out=slot(j), in_=src(i)).then_inc(slot_sem[j], 16)
            sync.wait_ge(act_done, 1)
            sync.dma_start(out=oav, in_=ob.ap().rearrange("p (g j) -> p g j", g=PG)).then_inc(out_sem, 16)
            sync.wait_ge(out_sem, 16)

        @block.vector
        def _(vector):
            for i in range(NTILES):
                j = i % NBUF
                g, hb = divmod(i, HO)
                vector.wait_ge(slot_sem[j], 16 * (i // NBUF + 1))
                tv = slot(j).rearrange("p (h pw wi) -> p pw h wi", h=BH, pw=WO, wi=BW)
                col = g * HO * WO + hb * WO
                vector.reduce_sum(
                    out=ob[:, col:col + WO], in_=tv, axis=mybir.AxisListType.XY
                ).then_inc(red_done, 1)

        @block.scalar
        def _(scalar):
            scalar.wait_ge(red_done, NTILES)
            scalar.mul(ob[:], ob[:], 1.0 / (BH * BW)).then_inc(act_done, 1)

    return nc


def get_nc():
    if "nc" not in _CACHE:
        _CACHE["nc"] = build_nc()
    return _CACHE["nc"]


def shard_inputs(x):
    x = np.asarray(x, dtype=np.float32).reshape(N_CORES, IMGS, H * W)
    return [{"x": np.ascontiguousarray(x[i])} for i in range(N_CORES)]


def kernel(x, H_in=224, W_in=224, **_):
    assert int(H_in) == H and int(W_in) == W
    res = run_bass_kernel_spmd(get_nc(), shard_inputs(x), core_ids=list(range(N_CORES)))
    out = np.stack([np.asarray(res.results[i]["out"]) for i in range(N_CORES)])
    return out.reshape(B, C, HO, WO)
